# revision 41
# baseline (speedup 1.0000x reference)
"""Chamfer-distance (bidirectional exact 1-NN) Trainium2 Bass kernel.

Problem: xyz1, xyz2 of shape [8, 4096, 3] fp32. For every point in cloud 1
find min/argmin squared distance to cloud 2 (dist1/idx1) and vice versa
(dist2/idx2), per batch.

Sharding: data-parallel over batch -- core b handles batch b (B=8=n_cores).
No cross-core communication.

Device algorithm per core (one batch), per direction:

1. Selection metric s[n,m] = 2*q_n.r_m - |r_m|^2 (= -d[n,m] + |q_n|^2; the
   q^2 term is independent of m so argmax_m s = argmin_m d). It is computed
   on the TensorEngine as K=27 bf16 matmuls ([128, 512] blocks): fp32 runs
   at 4 cycles/row on the PE but bf16 runs at 1, so each fp32 operand is
   split exactly into three bf16 limbs (x = h + m + l, Dekker style,
   lossless for fp32) and the six significant limb products (hh', hm',
   mh', hl', lh', mm') are stacked along the contraction dim (3 coords x 6
   = 18 rows). |r|^2 enters as 9 more rows: per-coordinate squares,
   computed and limb-split on device, paired against -1 rows. Dropped limb
   products are O(2^-24) relative -- the metric is fp32-class accurate and
   the PE runs 4x faster than an fp32 matmul.

2. The per-row argmax over the 4096 metric columns is split across two
   engines so neither is the bottleneck (DVE alone caps at 2 elem/cycle
   /lane @0.96GHz = 136us for both directions; the split gets under the
   PE roofline):
   - cols [0, 2048) "direct": two DVE pair-argmax custom ops, each
     streaming adjacent column pairs (2j, 2j+1) through both DVE read
     ports straight from PSUM (512 pairs per op).
   - cols [2048, 4096) "pooled": the Pool engine max-reduces adjacent
     column pairs into SBUF (2 ops, 512 wide), then one DVE pair-argmax
     op scans the two pooled arrays against each other (512 wide).
   Each op's accumulator output lands directly in a staged [128, 32]
   array slot -- no per-tile post-processing on any engine. PSUM is
   consumed in [128, 1024] sub-tiles (4 x 2 banks, bufs=4) so each
   consumer frees its banks independently and the PE never stalls.

3. This yields 4 winning blocks of 2 adjacent columns each per query
   (8 candidate reference points). Candidate blocks for all 32 tiles are
   fetched with 4 batched indirect DMAs per direction (SWDGE cost is
   ~1us fixed per call + 0.34ns/descriptor; batching 64 per-tile gathers
   into 4 amortizes the fixed cost away). Exact squared distances to the
   8 candidates are recomputed in full fp32 ((q-r)*(r-q) summed = -dist,
   bit-exact) and the min (value and lowest achieving column, matching
   argmin-first tie preference at the candidate level) selected with
   batched vector ops.

Host prep is lossless re-encoding only: batch slicing, transposes, and the
exact 3-limb bf16 split / doubling of input coordinates (h+m+l == x
bitwise). All arithmetic -- squares, their limb splits, distances,
argmins -- runs on device.

Argmax tie-breaking: the pair op returns the LAST pair achieving the
running maximum; jnp.argmin returns the first. They differ only when two
reference points have bit-identical fp32 metric values -- vanishingly rare
(fp32 near-ties already flip ~0.2% of indices between ANY two
arithmetically different implementations, including reference vs float64).
"""

import numpy as np

B = 8
N = 4096
P = 128
NT = N // P  # 32 query tiles
CH = 512     # matmul free-dim chunk (one PSUM bank)
DSUB = 1024  # metric columns per PSUM sub-tile (2 banks)

# limb-pair pattern: row j of each coordinate block pairs lhs limb L[j]
# with (doubled) rhs limb R[j]; products cover hh, hm, mh, hl, lh, mm.
_LHS_LIMB = [0, 0, 1, 0, 2, 1]  # 0=h 1=m 2=l
_RHS_LIMB = [0, 1, 0, 2, 0, 1]

_CACHE = {}


def _register_custom_ops():
    """Register the pair-argmax custom DVE op (idempotent)."""
    import concourse.dve_ops as dve_ops
    from concourse.dve_spec import (
        AluOp,
        C0,
        MaxNeg,
        One,
        Spec,
        Src0,
        Src1,
        eq,
        lower,
        maxx,
        scan,
        select,
    )
    from concourse.dve_uop import DveOpSpec

    def _register(name, spec):
        if name in dve_ops._SUB_OPCODE_FOR_NAME:
            return next(o for o in dve_ops.OPS if o.name == name)
        row = dve_ops._CUSTOM_DVE_ROW_BASE + len(dve_ops.OPS)
        assert row < 0x20, "custom DVE opcode rows exhausted"
        dve_ops._SUB_OPCODE_FOR_NAME[name] = row
        op = dve_ops.DveOp(name, spec, subdim=False, uops_sha={})
        for ver in ("v3", "v4"):
            compiled = DveOpSpec(
                name=name,
                opcode=row,
                uops=lower(spec, ver=ver),
                rd1_en=dve_ops.has_src1(spec),
            )
            op.uops_sha[ver] = compiled.sha(ver)
        dve_ops.OPS.append(op)
        dve_ops.CUSTOM_DVE_SPECS[name] = spec
        return op

    # Pair argmax: Src0/Src1 stream two element streams through both read
    # ports (2 metric elements per DVE cycle). Reports 2*j of the LAST pair
    # whose pairwise max achieves the running max; which member won is
    # resolved later by exact distance comparison. s0 must be -2.0 (index
    # scan seed).
    def argmax2_ref(in0, in1, c0, c1, c2):
        pm = np.maximum(in0, in1)
        r = np.maximum.accumulate(pm, axis=-1)
        ii = 2.0 * np.arange(in0.shape[-1], dtype=np.float32)
        body = np.where(pm == r, ii, -np.finfo(np.float32).max)
        return body, body.max(axis=-1, keepdims=True)

    _pm = maxx(Src0, Src1)
    argmax2_op = _register(
        "ARGMAX2_PAIR_ANT",
        Spec(
            body=select(
                eq(_pm, scan(AluOp.MAX, _pm)),
                scan(AluOp.ADD, One + One, init=C0),
                MaxNeg,
            ),
            accum=AluOp.MAX,
            reference=argmax2_ref,
        ),
    )
    return argmax2_op


def _build_program():
    from contextlib import ExitStack

    import concourse.bacc as bacc
    import concourse.bass as bass
    import concourse.mybir as mybir
    import concourse.tile as tile

    dt = mybir.dt
    Act = mybir.ActivationFunctionType
    Alu = mybir.AluOpType

    argmax2_op = _register_custom_ops()

    nc = bacc.Bacc("TRN2", target_bir_lowering=False, debug=False)

    # limb-expanded coordinate rows (host: lossless 3-limb bf16 split)
    qlh = nc.dram_tensor("qlh", [27, N], dt.bfloat16, kind="ExternalInput")
    qrh = nc.dram_tensor("qrh", [18, N], dt.bfloat16, kind="ExternalInput")
    rlh = nc.dram_tensor("rlh", [27, N], dt.bfloat16, kind="ExternalInput")
    rrh = nc.dram_tensor("rrh", [18, N], dt.bfloat16, kind="ExternalInput")
    # coords reshaped [48, 256]: partition 16k+a = coord k, sixteenth a --
    # setup ops use 48 partitions instead of 3 (16x fewer cycles, and the
    # square/limb-split dependency chain shortens to ~4us).
    q3 = nc.dram_tensor("q3", [48, N // 16], dt.float32, kind="ExternalInput")
    r3 = nc.dram_tensor("r3", [48, N // 16], dt.float32, kind="ExternalInput")
    qn = nc.dram_tensor("qn", [N, 3], dt.float32, kind="ExternalInput")
    rn = nc.dram_tensor("rn", [N, 3], dt.float32, kind="ExternalInput")
    # candidate-block re-encodings (see resolution below): row j holds the
    # points of one op's candidate columns back to back, so one gather
    # descriptor fetches the whole set.
    #   A: (pts[j], pts[1024+j])       -- op1 = (T1 psum, copy(T0) sbuf)
    #   B: (pts[2048+j], pts[3072+j])  -- op2 = (T3 psum, copy(T2) sbuf)
    blks = {}
    for cl in ("q", "r"):
        blks[cl] = (
            nc.dram_tensor(f"{cl}ba", [DSUB, 6], dt.float32, kind="ExternalInput"),
            nc.dram_tensor(f"{cl}bb", [DSUB, 6], dt.float32, kind="ExternalInput"),
        )
    d1 = nc.dram_tensor("d1", [P, NT], dt.float32, kind="ExternalOutput")
    d2 = nc.dram_tensor("d2", [P, NT], dt.float32, kind="ExternalOutput")
    i1 = nc.dram_tensor("i1", [P, NT], dt.int32, kind="ExternalOutput")
    i2 = nc.dram_tensor("i2", [P, NT], dt.int32, kind="ExternalOutput")

    with tile.TileContext(nc) as tc, ExitStack() as ctx:
        fixed = ctx.enter_context(tc.tile_pool(name="fixed", bufs=1))
        psum = ctx.enter_context(tc.tile_pool(name="psum", bufs=1, space="PSUM"))
        pmp = ctx.enter_context(tc.tile_pool(name="pmp", bufs=4))

        lhsq = fixed.tile([27, N], dt.bfloat16)  # q limbs + -1 rows
        rhsq = fixed.tile([27, N], dt.bfloat16)  # 2q limbs + |q|^2 limb rows
        lhsr = fixed.tile([27, N], dt.bfloat16)
        rhsr = fixed.tile([27, N], dt.bfloat16)
        scr = fixed.tile([P, 2 * DSUB], dt.float32)  # custom-op mandatory outs

        nc.sync.dma_start(lhsq[:], qlh.ap())
        nc.sync.dma_start(rhsq[0:18, :], qrh.ap())
        nc.sync.dma_start(lhsr[:], rlh.ap())
        nc.sync.dma_start(rhsr[0:18, :], rrh.ap())

        # PE p-state warmup: the TensorEngine runs at 0.65/1.2 GHz until it
        # has been busy ~3us, reaching 2.4 GHz after.  Spin it on junk
        # matmuls during the setup chain so the first real tiles run at
        # full clock.  (Input is a memset tile; output slot is the mm0 psum
        # slot, freed when its writers complete -- no reader needed.)
        warm = fixed.tile([27, 640], dt.bfloat16)
        nc.vector.memset(warm[:], 0.0)
        pswarm = psum.tile([P, DSUB], dt.float32, tag="mm0")
        for _ in range(12):
            nc.tensor.matmul(
                pswarm[:, 0:CH], warm[:, 0:P], warm[:, P:P + CH],
                start=True, stop=True,
            )

        # per-coordinate squares of each cloud, limb-split on device, into
        # rows 18..26 of that cloud's rhs tile (paired against the -1 rows).
        # r first: direction A (lhsq x rhsr) is scheduled first and only
        # needs rhsr's square rows; q's chain overlaps direction A compute.
        dcmp_cm = tc.tile_pool(name="dcmp", bufs=1)
        dcmp = dcmp_cm.__enter__()
        NQ = N // 16
        for c3d, rhs_t in ((r3, rhsr), (q3, rhsq)):
            c3 = dcmp.tile([48, NQ], dt.float32, tag="c3")
            nc.sync.dma_start(c3[:], c3d.ap())
            sqf = dcmp.tile([48, NQ], dt.float32, tag="sqf")
            nc.scalar.activation(sqf[:], c3[:], Act.Square)
            hh = dcmp.tile([48, NQ], dt.bfloat16, tag="hh")
            nc.scalar.copy(hh[:], sqf[:])
            t1 = dcmp.tile([48, NQ], dt.float32, tag="t1")
            nc.gpsimd.tensor_tensor(t1[:], sqf[:], hh[:], Alu.subtract)
            mm = dcmp.tile([48, NQ], dt.bfloat16, tag="mm")
            nc.scalar.copy(mm[:], t1[:])
            t2 = dcmp.tile([48, NQ], dt.float32, tag="t2")
            nc.gpsimd.tensor_tensor(t2[:], t1[:], mm[:], Alu.subtract)
            ll = dcmp.tile([48, NQ], dt.bfloat16, tag="ll")
            nc.scalar.copy(ll[:], t2[:])
            # row 18+3k+p of rhs = limb p of coord k: the 16 chunks of
            # coord k sit on contiguous partitions 16k..16k+15 -> one DMA.
            for k in range(3):
                for p, part in enumerate((hh, mm, ll)):
                    nc.sync.dma_start(
                        rhs_t[18 + 3 * k + p:19 + 3 * k + p, :].rearrange(
                            "r (a c) -> r a c", a=16
                        ),
                        part[16 * k:16 * k + 16, :],
                    )

        dcmp_cm.__exit__(None, None, None)

        # candidate-position weights 4..1 (lowest-index tie-break in the
        # final 4-way argmin): wt4f[p, s] = 4 - s
        wt4i = fixed.tile([P, 4], dt.int32)
        nc.gpsimd.iota(wt4i[:], pattern=[[-1, 4]], base=4, channel_multiplier=0)
        wt4f = fixed.tile([P, 4], dt.float32)
        nc.vector.tensor_copy(wt4f[:], wt4i[:])

        # per-direction staging (x2 so direction A's resolve overlaps B)
        def dir_tiles(di):
            t32f = [P, NT]
            return {
                "af": [
                    fixed.tile(t32f, dt.float32, name=f"af{k}_{di}")
                    for k in range(2)
                ],
                "qt4": fixed.tile([P, NT * 12], dt.float32, name=f"qt4_{di}"),
                "qt1": fixed.tile([P, NT * 3], dt.float32, name=f"qt1_{di}"),
                "rg": fixed.tile([P, NT * 12], dt.float32, name=f"rg_{di}"),
                "dfb": fixed.tile([P, NT * 12], dt.float32, name=f"dfb_{di}"),
                "d4n": fixed.tile([P, NT * 4], dt.float32, name=f"d4n_{di}"),
                "ge": fixed.tile([P, NT * 4], dt.float32, name=f"ge_{di}"),
                "dmax": fixed.tile(t32f, dt.float32, name=f"dmax_{di}"),
                "stg_d": fixed.tile(t32f, dt.float32, name=f"stgd_{di}"),
                "stg_i": fixed.tile(t32f, dt.uint32, name=f"stgi_{di}"),
                "off": [
                    fixed.tile(t32f, dt.uint32, name=f"off{k}_{di}")
                    for k in range(2)
                ],
                "sm": [
                    fixed.tile(t32f, dt.float32, name=f"sm{k}_{di}")
                    for k in range(5)
                ],
            }

        dirs = [dir_tiles(0), dir_tiles(1)]

        for di, (lhs_t, rhs_t, pts_blks, q_pts, dd, ii) in enumerate((
            (lhsq, rhsr, blks["r"], qn, d1, i1),
            (lhsr, rhsq, blks["q"], rn, d2, i2),
        )):
            g = dirs[di]
            af1, af2 = g["af"]
            rgt = g["rg"][:].rearrange("p (t x) -> p t x", x=12)
            # query points for the resolve: ONE strided load (the scheduler
            # hoists dep-free DMAs to the program start, so keep the DMA
            # fabric cost there minimal), replicated 4x on-chip by Act to
            # match the rg slot layout: qt4[p, 12t+3e+c] = q_pts[128t+p, c].
            qt4v = g["qt4"][:].rearrange("p (t x) -> p t x", x=12)
            qt1v = g["qt1"][:].rearrange("p (t x) -> p t x", x=3)
            qsrc = q_pts.ap().rearrange("(t p) c -> p t c", p=P)
            nc.sync.dma_start(qt1v[:], qsrc)
            for e in range(4):
                nc.scalar.copy(qt4v[:, :, 3 * e:3 * e + 3], qt1v[:])
            # ---- batched resolution, in two halves ----
            # af = 2j for each op; candidate columns per slot s:
            #   op1 (slots 0-1): j, 1024+j     (sbuf side first, then in0)
            #   op2 (slots 2-3): 2048+j, 3072+j
            # Block array A/B row j holds those two points (24B).
            # exact NEGATED squared distances to all 4 candidates:
            # (q-r)*(r-q) summed over coords == -(dist), bit-exact.
            # Emitted per half (tiles [0,16) mid-tile-loop at t=23 when its
            # gathers are long done, [16,32) at the end) so only half the
            # resolve chain sits in the direction tail.
            def emit_resolve(lo, hi):
                w = hi - lo
                rgs = g["rg"][:, 12 * lo:12 * hi]
                qts = g["qt4"][:, 12 * lo:12 * hi]
                dfs = g["dfb"][:, 12 * lo:12 * hi]
                d4s = g["d4n"][:, 4 * lo:4 * hi]
                ges = g["ge"][:, 4 * lo:4 * hi]
                dms = g["dmax"][:, lo:hi]
                nc.gpsimd.tensor_tensor(dfs, rgs, qts, Alu.subtract)
                nc.gpsimd.tensor_tensor(rgs, qts, rgs, Alu.subtract)
                nc.gpsimd.tensor_tensor(rgs, rgs, dfs, Alu.mult)
                nc.vector.tensor_reduce(
                    d4s, rgs.rearrange("p (g c) -> p g c", c=3),
                    axis=mybir.AxisListType.X, op=Alu.add,
                )
                # min distance = -max(negated); winning candidate = lowest
                # slot achieving it (wt4f is 4-s, descending).
                d4v = d4s.rearrange("p (t e) -> p t e", e=4)
                nc.vector.tensor_reduce(
                    dms, d4v, axis=mybir.AxisListType.X, op=Alu.max
                )
                gev = ges.rearrange("p (t e) -> p t e", e=4)
                dmax_b = dms.unsqueeze(2).broadcast_to((P, w, 4))
                nc.vector.tensor_tensor(gev, d4v, dmax_b, Alu.is_ge)
                wt_b = wt4f[:].unsqueeze(1).broadcast_to((P, w, 4))
                nc.vector.tensor_tensor(gev, gev, wt_b, Alu.mult)
                cc, acc, gm2, tmp, tmp2 = [s[:, lo:hi] for s in g["sm"]]
                nc.vector.tensor_reduce(
                    cc, gev, axis=mybir.AxisListType.X, op=Alu.max
                )
                nc.gpsimd.tensor_scalar(
                    g["stg_d"][:, lo:hi], dms, -1.0, None, Alu.mult
                )
                nc.sync.dma_start(dd.ap()[:, lo:hi], g["stg_d"][:, lo:hi])
                # winning slot s = 4 - wmax in [0,4); column =
                #   af_sel/2 + 1024*(g1+g2+g3), af_sel = af1 if s<2 else af2.
                # All [P, w]-wide; runs on GPSIMD to keep DVE free.
                nc.gpsimd.tensor_scalar(cc, cc, -1.0, 4.0, Alu.mult, Alu.add)
                nc.gpsimd.tensor_scalar(acc, cc, 1.0, 1024.0, Alu.is_ge, Alu.mult)
                nc.gpsimd.tensor_scalar(tmp, cc, 2.0, 1024.0, Alu.is_ge, Alu.mult)
                nc.gpsimd.tensor_tensor(acc, acc, tmp, Alu.add)
                nc.gpsimd.tensor_scalar(tmp, cc, 3.0, 1024.0, Alu.is_ge, Alu.mult)
                nc.gpsimd.tensor_tensor(acc, acc, tmp, Alu.add)
                nc.gpsimd.tensor_scalar(gm2, cc, 2.0, None, Alu.is_ge)
                nc.gpsimd.tensor_scalar(tmp, gm2, -1.0, 1.0, Alu.mult, Alu.add)
                nc.gpsimd.tensor_scalar(tmp2, af1[:, lo:hi], 0.5, None, Alu.mult)
                nc.gpsimd.tensor_tensor(tmp, tmp, tmp2, Alu.mult)
                nc.gpsimd.tensor_tensor(acc, acc, tmp, Alu.add)
                nc.gpsimd.tensor_scalar(tmp2, af2[:, lo:hi], 0.5, None, Alu.mult)
                nc.gpsimd.tensor_tensor(tmp, gm2, tmp2, Alu.mult)
                nc.gpsimd.tensor_tensor(acc, acc, tmp, Alu.add)
                nc.gpsimd.tensor_copy(g["stg_i"][:, lo:hi], acc)
                nc.sync.dma_start(
                    ii.ap()[:, lo:hi],
                    g["stg_i"][:, lo:hi].bitcast(dt.int32),
                )


            for t in range(NT):
                # metric sub-tiles T_k = cols [1024k, 1024k+1024).  PSUM can
                # only be read by Act (copies) and DVE (one operand per op):
                #   Act: sA = copy(T0), sB = copy(T2)
                #   DVE: op1 = pair-argmax(T1 psum, sA sbuf)   -> af1
                #        op2 = pair-argmax(T3 psum, sB sbuf)   -> af2
                ps_t = []
                for kk in range(4):
                    ps = psum.tile([P, DSUB], dt.float32, tag=f"mm{kk}")
                    for c in range(2):
                        nc.tensor.matmul(
                            ps[:, c * CH:(c + 1) * CH],
                            lhs_t[:, t * P:(t + 1) * P],
                            rhs_t[:, (2 * kk + c) * CH:(2 * kk + c + 1) * CH],
                            start=True,
                            stop=True,
                        )
                    ps_t.append(ps)

                sA = pmp.tile([P, DSUB], dt.float32, tag="sA", bufs=2)
                nc.scalar.copy(sA[:], ps_t[0][:])
                nc.vector._custom_dve(
                    argmax2_op, out=scr[:, 0:DSUB],
                    in0=ps_t[1][:], in1=sA[:],
                    s0=-2.0, accum_out=af1[:, t:t + 1],
                )
                sB = pmp.tile([P, DSUB], dt.float32, tag="sB", bufs=2)
                nc.scalar.copy(sB[:], ps_t[2][:])
                nc.vector._custom_dve(
                    argmax2_op, out=scr[:, DSUB:2 * DSUB],
                    in0=ps_t[3][:], in1=sB[:],
                    s0=-2.0, accum_out=af2[:, t:t + 1],
                )

                batch = (t % 4 == 3) if t < 24 else (t % 2 == 1)
                if batch:
                    # convert the last tiles' accumulators to gather row
                    # offsets (af = 2j -> row j), then fetch each tile's two
                    # candidate blocks (24B each) with [P, 1]-offset gathers
                    # (the SWDGE services one offset per partition per call).
                    # Batches shrink near the direction end so the final
                    # gathers finish right after the last argmax op instead
                    # of queueing 8 deep behind it.
                    t0 = t - 3 if t < 24 else t - 1
                    for c in range(2):
                        u = g["off"][c]
                        nc.vector.tensor_copy(
                            u[:, t0:t + 1], g["af"][c][:, t0:t + 1]
                        )
                        nc.vector.tensor_scalar(
                            u[:, t0:t + 1], u[:, t0:t + 1], 1, None,
                            Alu.logical_shift_right,
                        )
                    for tg in range(t0, t + 1):
                        for c, blk in enumerate(pts_blks):
                            nc.gpsimd.indirect_dma_start(
                                out=rgt[:, tg, 6 * c:6 * c + 6],
                                out_offset=None,
                                in_=blk.ap(),
                                in_offset=bass.IndirectOffsetOnAxis(
                                    ap=g["off"][c][:, tg:tg + 1], axis=0
                                ),
                            )

                if t == 23:
                    emit_resolve(0, 16)

            emit_resolve(16, NT)

    # Bacc compile legalizes multi-wait instructions (walrus accepts only a
    # single sync wait per instruction) via nop chains, plus DCE/nop-fusion.
    nc.compile()
    return nc


def _get_program():
    if "nc" not in _CACHE:
        _CACHE["nc"] = _build_program()
    return _CACHE["nc"]


def _limb_split(x):
    """Exact 3-limb bf16 split: x == h + m + l bitwise for fp32 input."""
    import ml_dtypes

    h = x.astype(ml_dtypes.bfloat16)
    res = x - h.astype(np.float32)
    m = res.astype(ml_dtypes.bfloat16)
    l = (res - m.astype(np.float32)).astype(ml_dtypes.bfloat16)
    return h, m, l


def _limb_rows(c3, doubled):
    """Build the bf16 limb-pattern rows for a [3, N] fp32 coord array.

    lhs pattern (doubled=False): [27, N] -- 18 limb rows plus 9 rows of -1
    (constant companions for the on-device |r|^2 limb rows).
    rhs pattern (doubled=True): [18, N] limb rows of 2*c3.
    """
    import ml_dtypes

    src = (c3 * 2.0) if doubled else c3
    limbs = _limb_split(src)  # tuple of three [3, N] bf16
    pattern = _RHS_LIMB if doubled else _LHS_LIMB
    nrows = 18 if doubled else 27
    out = np.full((nrows, c3.shape[1]), -1.0, dtype=ml_dtypes.bfloat16)
    for k in range(3):
        for j in range(6):
            out[6 * k + j] = limbs[pattern[j]][k]
    return out


def _cand_blocks(pts):
    """[4096, 3] -> candidate-pair block arrays A, B [1024, 6]; row j of A =
    (pts[j], pts[1024+j]), row j of B = (pts[2048+j], pts[3072+j])."""
    c = pts.reshape(4, DSUB, 3)
    A = np.concatenate([c[0], c[1]], axis=1)
    Bb = np.concatenate([c[2], c[3]], axis=1)
    return (np.ascontiguousarray(A), np.ascontiguousarray(Bb))


def make_in_maps(xyz1, xyz2):
    xyz1 = np.asarray(xyz1, dtype=np.float32)
    xyz2 = np.asarray(xyz2, dtype=np.float32)
    in_maps = []
    for b in range(B):
        q3 = np.ascontiguousarray(xyz1[b].T)
        r3 = np.ascontiguousarray(xyz2[b].T)
        # [3, N] -> [48, N//16]: partition 16k+a = coord k, sixteenth a
        q3q = np.ascontiguousarray(q3.reshape(48, N // 16))
        r3q = np.ascontiguousarray(r3.reshape(48, N // 16))
        in_maps.append(
            {
                "qlh": _limb_rows(q3, doubled=False),
                "qrh": _limb_rows(q3, doubled=True),
                "rlh": _limb_rows(r3, doubled=False),
                "rrh": _limb_rows(r3, doubled=True),
                "q3": q3q,
                "r3": r3q,
                "qn": np.ascontiguousarray(xyz1[b]),
                "rn": np.ascontiguousarray(xyz2[b]),
            }
        )
        for cl, pts in (("q", xyz1[b]), ("r", xyz2[b])):
            for sfx, arr in zip("ab", _cand_blocks(pts)):
                in_maps[-1][f"{cl}b{sfx}"] = arr
    return in_maps


def unpack_outputs(results):
    d1 = np.stack([results[b]["d1"].T.reshape(-1) for b in range(B)])
    d2 = np.stack([results[b]["d2"].T.reshape(-1) for b in range(B)])
    i1 = np.stack([results[b]["i1"].T.reshape(-1) for b in range(B)])
    i2 = np.stack([results[b]["i2"].T.reshape(-1) for b in range(B)])
    return (
        d1.astype(np.float32),
        d2.astype(np.float32),
        i1.astype(np.int32),
        i2.astype(np.int32),
    )


def kernel(xyz1, xyz2):
    from concourse.bass_utils import run_bass_kernel_spmd

    nc = _get_program()
    in_maps = make_in_maps(xyz1, xyz2)
    res = run_bass_kernel_spmd(nc, in_maps, core_ids=list(range(B)))
    _CACHE["last_results"] = res
    return unpack_outputs(res.results)


# revision 42
# speedup vs baseline: 1.0204x; 1.0204x over previous
"""Chamfer-distance (bidirectional exact 1-NN) Trainium2 Bass kernel.

Problem: xyz1, xyz2 of shape [8, 4096, 3] fp32. For every point in cloud 1
find min/argmin squared distance to cloud 2 (dist1/idx1) and vice versa
(dist2/idx2), per batch.

Sharding: data-parallel over batch -- core b handles batch b (B=8=n_cores).
No cross-core communication.

Device algorithm per core (one batch), per direction:

1. Selection metric s[n,m] = 2*q_n.r_m - |r_m|^2 (= -d[n,m] + |q_n|^2; the
   q^2 term is independent of m so argmax_m s = argmin_m d). It is computed
   on the TensorEngine as K=27 bf16 matmuls ([128, 512] blocks): fp32 runs
   at 4 cycles/row on the PE but bf16 runs at 1, so each fp32 operand is
   split exactly into three bf16 limbs (x = h + m + l, Dekker style,
   lossless for fp32) and the six significant limb products (hh', hm',
   mh', hl', lh', mm') are stacked along the contraction dim (3 coords x 6
   = 18 rows). |r|^2 enters as 9 more rows: per-coordinate squares,
   computed and limb-split on device, paired against -1 rows. Dropped limb
   products are O(2^-24) relative -- the metric is fp32-class accurate and
   the PE runs 4x faster than an fp32 matmul.

2. The per-row argmax over the 4096 metric columns is split across two
   engines so neither is the bottleneck (DVE alone caps at 2 elem/cycle
   /lane @0.96GHz = 136us for both directions; the split gets under the
   PE roofline):
   - cols [0, 2048) "direct": two DVE pair-argmax custom ops, each
     streaming adjacent column pairs (2j, 2j+1) through both DVE read
     ports straight from PSUM (512 pairs per op).
   - cols [2048, 4096) "pooled": the Pool engine max-reduces adjacent
     column pairs into SBUF (2 ops, 512 wide), then one DVE pair-argmax
     op scans the two pooled arrays against each other (512 wide).
   Each op's accumulator output lands directly in a staged [128, 32]
   array slot -- no per-tile post-processing on any engine. PSUM is
   consumed in [128, 1024] sub-tiles (4 x 2 banks, bufs=4) so each
   consumer frees its banks independently and the PE never stalls.

3. This yields 4 winning blocks of 2 adjacent columns each per query
   (8 candidate reference points). Candidate blocks for all 32 tiles are
   fetched with 4 batched indirect DMAs per direction (SWDGE cost is
   ~1us fixed per call + 0.34ns/descriptor; batching 64 per-tile gathers
   into 4 amortizes the fixed cost away). Exact squared distances to the
   8 candidates are recomputed in full fp32 ((q-r)*(r-q) summed = -dist,
   bit-exact) and the min (value and lowest achieving column, matching
   argmin-first tie preference at the candidate level) selected with
   batched vector ops.

Host prep is lossless re-encoding only: batch slicing, transposes, and the
exact 3-limb bf16 split / doubling of input coordinates (h+m+l == x
bitwise). All arithmetic -- squares, their limb splits, distances,
argmins -- runs on device.

Argmax tie-breaking: the pair op returns the LAST pair achieving the
running maximum; jnp.argmin returns the first. They differ only when two
reference points have bit-identical fp32 metric values -- vanishingly rare
(fp32 near-ties already flip ~0.2% of indices between ANY two
arithmetically different implementations, including reference vs float64).
"""

import numpy as np

B = 8
N = 4096
P = 128
NT = N // P  # 32 query tiles
CH = 512     # matmul free-dim chunk (one PSUM bank)
DSUB = 1024  # metric columns per PSUM sub-tile (2 banks)

# limb-pair pattern: row j of each coordinate block pairs lhs limb L[j]
# with (doubled) rhs limb R[j]; products cover hh, hm, mh, hl, lh, mm.
_LHS_LIMB = [0, 0, 1, 0, 2, 1]  # 0=h 1=m 2=l
_RHS_LIMB = [0, 1, 0, 2, 0, 1]

_CACHE = {}


def _register_custom_ops():
    """Register the pair-argmax custom DVE op (idempotent)."""
    import concourse.dve_ops as dve_ops
    from concourse.dve_spec import (
        AluOp,
        C0,
        MaxNeg,
        One,
        Spec,
        Src0,
        Src1,
        eq,
        lower,
        maxx,
        scan,
        select,
    )
    from concourse.dve_uop import DveOpSpec

    def _register(name, spec):
        if name in dve_ops._SUB_OPCODE_FOR_NAME:
            return next(o for o in dve_ops.OPS if o.name == name)
        row = dve_ops._CUSTOM_DVE_ROW_BASE + len(dve_ops.OPS)
        assert row < 0x20, "custom DVE opcode rows exhausted"
        dve_ops._SUB_OPCODE_FOR_NAME[name] = row
        op = dve_ops.DveOp(name, spec, subdim=False, uops_sha={})
        for ver in ("v3", "v4"):
            compiled = DveOpSpec(
                name=name,
                opcode=row,
                uops=lower(spec, ver=ver),
                rd1_en=dve_ops.has_src1(spec),
            )
            op.uops_sha[ver] = compiled.sha(ver)
        dve_ops.OPS.append(op)
        dve_ops.CUSTOM_DVE_SPECS[name] = spec
        return op

    # Pair argmax: Src0/Src1 stream two element streams through both read
    # ports (2 metric elements per DVE cycle). Reports 2*j of the LAST pair
    # whose pairwise max achieves the running max; which member won is
    # resolved later by exact distance comparison. s0 must be -2.0 (index
    # scan seed).
    def argmax2_ref(in0, in1, c0, c1, c2):
        pm = np.maximum(in0, in1)
        r = np.maximum.accumulate(pm, axis=-1)
        ii = 2.0 * np.arange(in0.shape[-1], dtype=np.float32)
        body = np.where(pm == r, ii, -np.finfo(np.float32).max)
        return body, body.max(axis=-1, keepdims=True)

    _pm = maxx(Src0, Src1)
    argmax2_op = _register(
        "ARGMAX2_PAIR_ANT",
        Spec(
            body=select(
                eq(_pm, scan(AluOp.MAX, _pm)),
                scan(AluOp.ADD, One + One, init=C0),
                MaxNeg,
            ),
            accum=AluOp.MAX,
            reference=argmax2_ref,
        ),
    )
    return argmax2_op


def _build_program():
    from contextlib import ExitStack

    import concourse.bacc as bacc
    import concourse.bass as bass
    import concourse.mybir as mybir
    import concourse.tile as tile

    dt = mybir.dt
    Act = mybir.ActivationFunctionType
    Alu = mybir.AluOpType

    argmax2_op = _register_custom_ops()

    nc = bacc.Bacc("TRN2", target_bir_lowering=False, debug=False)

    # limb-expanded coordinate rows (host: lossless 3-limb bf16 split)
    qlh = nc.dram_tensor("qlh", [27, N], dt.bfloat16, kind="ExternalInput")
    qrh = nc.dram_tensor("qrh", [18, N], dt.bfloat16, kind="ExternalInput")
    rlh = nc.dram_tensor("rlh", [27, N], dt.bfloat16, kind="ExternalInput")
    rrh = nc.dram_tensor("rrh", [18, N], dt.bfloat16, kind="ExternalInput")
    # coords reshaped [48, 256]: partition 16k+a = coord k, sixteenth a --
    # setup ops use 48 partitions instead of 3 (16x fewer cycles, and the
    # square/limb-split dependency chain shortens to ~4us).
    q3 = nc.dram_tensor("q3", [48, N // 16], dt.float32, kind="ExternalInput")
    r3 = nc.dram_tensor("r3", [48, N // 16], dt.float32, kind="ExternalInput")
    qn = nc.dram_tensor("qn", [N, 3], dt.float32, kind="ExternalInput")
    rn = nc.dram_tensor("rn", [N, 3], dt.float32, kind="ExternalInput")
    # candidate-block re-encodings (see resolution below): row j holds the
    # points of one op's candidate columns back to back, so one gather
    # descriptor fetches the whole set.
    #   A: (pts[j], pts[1024+j])       -- op1 = (T1 psum, copy(T0) sbuf)
    #   B: (pts[2048+j], pts[3072+j])  -- op2 = (T3 psum, copy(T2) sbuf)
    blks = {}
    for cl in ("q", "r"):
        blks[cl] = (
            nc.dram_tensor(f"{cl}ba", [DSUB, 6], dt.float32, kind="ExternalInput"),
            nc.dram_tensor(f"{cl}bb", [DSUB, 6], dt.float32, kind="ExternalInput"),
        )
    d1 = nc.dram_tensor("d1", [P, NT], dt.float32, kind="ExternalOutput")
    d2 = nc.dram_tensor("d2", [P, NT], dt.float32, kind="ExternalOutput")
    i1 = nc.dram_tensor("i1", [P, NT], dt.int32, kind="ExternalOutput")
    i2 = nc.dram_tensor("i2", [P, NT], dt.int32, kind="ExternalOutput")

    with tile.TileContext(nc) as tc, ExitStack() as ctx:
        fixed = ctx.enter_context(tc.tile_pool(name="fixed", bufs=1))
        psum = ctx.enter_context(tc.tile_pool(name="psum", bufs=1, space="PSUM"))
        pmp = ctx.enter_context(tc.tile_pool(name="pmp", bufs=4))

        lhsq = fixed.tile([27, N], dt.bfloat16)  # q limbs + -1 rows
        rhsq = fixed.tile([27, N], dt.bfloat16)  # 2q limbs + |q|^2 limb rows
        lhsr = fixed.tile([27, N], dt.bfloat16)
        rhsr = fixed.tile([27, N], dt.bfloat16)
        scr = fixed.tile([P, 2 * DSUB], dt.float32)  # custom-op mandatory outs

        nc.sync.dma_start(lhsq[:], qlh.ap())
        nc.sync.dma_start(rhsq[0:18, :], qrh.ap())
        nc.sync.dma_start(lhsr[:], rlh.ap())
        nc.sync.dma_start(rhsr[0:18, :], rrh.ap())

        # PE p-state warmup: the TensorEngine runs at 0.65/1.2 GHz until it
        # has been busy ~3us, reaching 2.4 GHz after.  Spin it on junk
        # matmuls during the setup chain so the first real tiles run at
        # full clock.  (Input is a memset tile; output slot is the mm0 psum
        # slot, freed when its writers complete -- no reader needed.)
        warm = fixed.tile([27, 640], dt.bfloat16)
        nc.vector.memset(warm[:], 0.0)
        pswarm = psum.tile([P, DSUB], dt.float32, tag="mm0")
        for _ in range(12):
            nc.tensor.matmul(
                pswarm[:, 0:CH], warm[:, 0:P], warm[:, P:P + CH],
                start=True, stop=True,
            )

        # per-coordinate squares of each cloud, limb-split on device, into
        # rows 18..26 of that cloud's rhs tile (paired against the -1 rows).
        # r first: direction A (lhsq x rhsr) is scheduled first and only
        # needs rhsr's square rows; q's chain overlaps direction A compute.
        dcmp_cm = tc.tile_pool(name="dcmp", bufs=1)
        dcmp = dcmp_cm.__enter__()
        NQ = N // 16
        for c3d, rhs_t in ((r3, rhsr), (q3, rhsq)):
            c3 = dcmp.tile([48, NQ], dt.float32, tag="c3")
            nc.sync.dma_start(c3[:], c3d.ap())
            sqf = dcmp.tile([48, NQ], dt.float32, tag="sqf")
            nc.scalar.activation(sqf[:], c3[:], Act.Square)
            hh = dcmp.tile([48, NQ], dt.bfloat16, tag="hh")
            nc.scalar.copy(hh[:], sqf[:])
            t1 = dcmp.tile([48, NQ], dt.float32, tag="t1")
            nc.gpsimd.tensor_tensor(t1[:], sqf[:], hh[:], Alu.subtract)
            mm = dcmp.tile([48, NQ], dt.bfloat16, tag="mm")
            nc.scalar.copy(mm[:], t1[:])
            t2 = dcmp.tile([48, NQ], dt.float32, tag="t2")
            nc.gpsimd.tensor_tensor(t2[:], t1[:], mm[:], Alu.subtract)
            ll = dcmp.tile([48, NQ], dt.bfloat16, tag="ll")
            nc.scalar.copy(ll[:], t2[:])
            # row 18+3k+p of rhs = limb p of coord k: the 16 chunks of
            # coord k sit on contiguous partitions 16k..16k+15 -> one DMA.
            for k in range(3):
                for p, part in enumerate((hh, mm, ll)):
                    nc.sync.dma_start(
                        rhs_t[18 + 3 * k + p:19 + 3 * k + p, :].rearrange(
                            "r (a c) -> r a c", a=16
                        ),
                        part[16 * k:16 * k + 16, :],
                    )

        dcmp_cm.__exit__(None, None, None)

        # candidate-position weights 4..1 (lowest-index tie-break in the
        # final 4-way argmin): wt4f[p, s] = 4 - s
        wt4i = fixed.tile([P, 4], dt.int32)
        nc.gpsimd.iota(wt4i[:], pattern=[[-1, 4]], base=4, channel_multiplier=0)
        wt4f = fixed.tile([P, 4], dt.float32)
        nc.vector.tensor_copy(wt4f[:], wt4i[:])

        # per-direction staging (x2 so direction A's resolve overlaps B)
        def dir_tiles(di):
            t32f = [P, NT]
            return {
                "af": [
                    fixed.tile(t32f, dt.float32, name=f"af{k}_{di}")
                    for k in range(2)
                ],
                "qt4": fixed.tile([P, NT * 12], dt.float32, name=f"qt4_{di}"),
                "qt1": fixed.tile([P, NT * 3], dt.float32, name=f"qt1_{di}"),
                "rg": fixed.tile([P, NT * 12], dt.float32, name=f"rg_{di}"),
                "dfb": fixed.tile([P, NT * 12], dt.float32, name=f"dfb_{di}"),
                "d4n": fixed.tile([P, NT * 4], dt.float32, name=f"d4n_{di}"),
                "ge": fixed.tile([P, NT * 4], dt.float32, name=f"ge_{di}"),
                "dmax": fixed.tile(t32f, dt.float32, name=f"dmax_{di}"),
                "stg_d": fixed.tile(t32f, dt.float32, name=f"stgd_{di}"),
                "stg_i": fixed.tile(t32f, dt.uint32, name=f"stgi_{di}"),
                "off": [
                    fixed.tile(t32f, dt.uint32, name=f"off{k}_{di}")
                    for k in range(2)
                ],
                "sm": [
                    fixed.tile(t32f, dt.float32, name=f"sm{k}_{di}")
                    for k in range(5)
                ],
            }

        dirs = [dir_tiles(0), dir_tiles(1)]

        for di, (lhs_t, rhs_t, pts_blks, q_pts, dd, ii) in enumerate((
            (lhsq, rhsr, blks["r"], qn, d1, i1),
            (lhsr, rhsq, blks["q"], rn, d2, i2),
        )):
            g = dirs[di]
            af1, af2 = g["af"]
            rgt = g["rg"][:].rearrange("p (t x) -> p t x", x=12)
            # query points for the resolve: ONE strided load (the scheduler
            # hoists dep-free DMAs to the program start, so keep the DMA
            # fabric cost there minimal), replicated 4x on-chip by Act to
            # match the rg slot layout: qt4[p, 12t+3e+c] = q_pts[128t+p, c].
            qt4v = g["qt4"][:].rearrange("p (t x) -> p t x", x=12)
            qt1v = g["qt1"][:].rearrange("p (t x) -> p t x", x=3)
            qsrc = q_pts.ap().rearrange("(t p) c -> p t c", p=P)
            nc.sync.dma_start(qt1v[:], qsrc)
            for e in range(4):
                nc.scalar.copy(qt4v[:, :, 3 * e:3 * e + 3], qt1v[:])
            # ---- batched resolution, in two halves ----
            # af = 2j for each op; candidate columns per slot s:
            #   op1 (slots 0-1): j, 1024+j     (sbuf side first, then in0)
            #   op2 (slots 2-3): 2048+j, 3072+j
            # Block array A/B row j holds those two points (24B).
            # exact NEGATED squared distances to all 4 candidates:
            # (q-r)*(r-q) summed over coords == -(dist), bit-exact.
            # Emitted per half (tiles [0,16) mid-tile-loop at t=23 when its
            # gathers are long done, [16,32) at the end) so only half the
            # resolve chain sits in the direction tail.
            def emit_resolve(lo, hi):
                w = hi - lo
                rgs = g["rg"][:, 12 * lo:12 * hi]
                qts = g["qt4"][:, 12 * lo:12 * hi]
                dfs = g["dfb"][:, 12 * lo:12 * hi]
                d4s = g["d4n"][:, 4 * lo:4 * hi]
                ges = g["ge"][:, 4 * lo:4 * hi]
                dms = g["dmax"][:, lo:hi]
                nc.vector.tensor_tensor(dfs, rgs, qts, Alu.subtract)
                nc.vector.tensor_tensor(rgs, qts, rgs, Alu.subtract)
                nc.vector.tensor_tensor(rgs, rgs, dfs, Alu.mult)
                nc.vector.tensor_reduce(
                    d4s, rgs.rearrange("p (g c) -> p g c", c=3),
                    axis=mybir.AxisListType.X, op=Alu.add,
                )
                # min distance = -max(negated); winning candidate = lowest
                # slot achieving it (wt4f is 4-s, descending).
                d4v = d4s.rearrange("p (t e) -> p t e", e=4)
                nc.vector.tensor_reduce(
                    dms, d4v, axis=mybir.AxisListType.X, op=Alu.max
                )
                gev = ges.rearrange("p (t e) -> p t e", e=4)
                dmax_b = dms.unsqueeze(2).broadcast_to((P, w, 4))
                nc.vector.tensor_tensor(gev, d4v, dmax_b, Alu.is_ge)
                wt_b = wt4f[:].unsqueeze(1).broadcast_to((P, w, 4))
                nc.vector.tensor_tensor(gev, gev, wt_b, Alu.mult)
                cc, acc, gm2, tmp, tmp2 = [s[:, lo:hi] for s in g["sm"]]
                nc.vector.tensor_reduce(
                    cc, gev, axis=mybir.AxisListType.X, op=Alu.max
                )
                nc.vector.tensor_scalar(
                    g["stg_d"][:, lo:hi], dms, -1.0, None, Alu.mult
                )
                nc.sync.dma_start(dd.ap()[:, lo:hi], g["stg_d"][:, lo:hi])
                # winning slot s = 4 - wmax in [0,4); column =
                #   af_sel/2 + 1024*(g1+g2+g3), af_sel = af1 if s<2 else af2.
                # All [P, w]-wide; runs on GPSIMD to keep DVE free.
                nc.gpsimd.tensor_scalar(cc, cc, -1.0, 4.0, Alu.mult, Alu.add)
                nc.gpsimd.tensor_scalar(acc, cc, 1.0, 1024.0, Alu.is_ge, Alu.mult)
                nc.gpsimd.tensor_scalar(tmp, cc, 2.0, 1024.0, Alu.is_ge, Alu.mult)
                nc.gpsimd.tensor_tensor(acc, acc, tmp, Alu.add)
                nc.gpsimd.tensor_scalar(tmp, cc, 3.0, 1024.0, Alu.is_ge, Alu.mult)
                nc.gpsimd.tensor_tensor(acc, acc, tmp, Alu.add)
                nc.gpsimd.tensor_scalar(gm2, cc, 2.0, None, Alu.is_ge)
                nc.gpsimd.tensor_scalar(tmp, gm2, -1.0, 1.0, Alu.mult, Alu.add)
                nc.gpsimd.tensor_scalar(tmp2, af1[:, lo:hi], 0.5, None, Alu.mult)
                nc.gpsimd.tensor_tensor(tmp, tmp, tmp2, Alu.mult)
                nc.gpsimd.tensor_tensor(acc, acc, tmp, Alu.add)
                nc.gpsimd.tensor_scalar(tmp2, af2[:, lo:hi], 0.5, None, Alu.mult)
                nc.gpsimd.tensor_tensor(tmp, gm2, tmp2, Alu.mult)
                nc.gpsimd.tensor_tensor(acc, acc, tmp, Alu.add)
                nc.gpsimd.tensor_copy(g["stg_i"][:, lo:hi], acc)
                nc.sync.dma_start(
                    ii.ap()[:, lo:hi],
                    g["stg_i"][:, lo:hi].bitcast(dt.int32),
                )


            for t in range(NT):
                # metric sub-tiles T_k = cols [1024k, 1024k+1024).  PSUM can
                # only be read by Act (copies) and DVE (one operand per op):
                #   Act: sA = copy(T0), sB = copy(T2)
                #   DVE: op1 = pair-argmax(T1 psum, sA sbuf)   -> af1
                #        op2 = pair-argmax(T3 psum, sB sbuf)   -> af2
                ps_t = []
                for kk in range(4):
                    ps = psum.tile([P, DSUB], dt.float32, tag=f"mm{kk}")
                    for c in range(2):
                        nc.tensor.matmul(
                            ps[:, c * CH:(c + 1) * CH],
                            lhs_t[:, t * P:(t + 1) * P],
                            rhs_t[:, (2 * kk + c) * CH:(2 * kk + c + 1) * CH],
                            start=True,
                            stop=True,
                        )
                    ps_t.append(ps)

                sA = pmp.tile([P, DSUB], dt.float32, tag="sA", bufs=2)
                nc.scalar.copy(sA[:], ps_t[0][:])
                nc.vector._custom_dve(
                    argmax2_op, out=scr[:, 0:DSUB],
                    in0=ps_t[1][:], in1=sA[:],
                    s0=-2.0, accum_out=af1[:, t:t + 1],
                )
                sB = pmp.tile([P, DSUB], dt.float32, tag="sB", bufs=2)
                nc.scalar.copy(sB[:], ps_t[2][:])
                nc.vector._custom_dve(
                    argmax2_op, out=scr[:, DSUB:2 * DSUB],
                    in0=ps_t[3][:], in1=sB[:],
                    s0=-2.0, accum_out=af2[:, t:t + 1],
                )

                batch = (t % 4 == 3) if t < 24 else (t % 2 == 1)
                if batch:
                    # convert the last tiles' accumulators to gather row
                    # offsets (af = 2j -> row j), then fetch each tile's two
                    # candidate blocks (24B each) with [P, 1]-offset gathers
                    # (the SWDGE services one offset per partition per call).
                    # Batches shrink near the direction end so the final
                    # gathers finish right after the last argmax op instead
                    # of queueing 8 deep behind it.
                    t0 = t - 3 if t < 24 else t - 1
                    for c in range(2):
                        u = g["off"][c]
                        nc.vector.tensor_copy(
                            u[:, t0:t + 1], g["af"][c][:, t0:t + 1]
                        )
                        nc.vector.tensor_scalar(
                            u[:, t0:t + 1], u[:, t0:t + 1], 1, None,
                            Alu.logical_shift_right,
                        )
                    for tg in range(t0, t + 1):
                        for c, blk in enumerate(pts_blks):
                            nc.gpsimd.indirect_dma_start(
                                out=rgt[:, tg, 6 * c:6 * c + 6],
                                out_offset=None,
                                in_=blk.ap(),
                                in_offset=bass.IndirectOffsetOnAxis(
                                    ap=g["off"][c][:, tg:tg + 1], axis=0
                                ),
                            )

                if t == 23:
                    emit_resolve(0, 16)

            emit_resolve(16, NT)

    # Bacc compile legalizes multi-wait instructions (walrus accepts only a
    # single sync wait per instruction) via nop chains, plus DCE/nop-fusion.
    nc.compile()
    return nc


def _get_program():
    if "nc" not in _CACHE:
        _CACHE["nc"] = _build_program()
    return _CACHE["nc"]


def _limb_split(x):
    """Exact 3-limb bf16 split: x == h + m + l bitwise for fp32 input."""
    import ml_dtypes

    h = x.astype(ml_dtypes.bfloat16)
    res = x - h.astype(np.float32)
    m = res.astype(ml_dtypes.bfloat16)
    l = (res - m.astype(np.float32)).astype(ml_dtypes.bfloat16)
    return h, m, l


def _limb_rows(c3, doubled):
    """Build the bf16 limb-pattern rows for a [3, N] fp32 coord array.

    lhs pattern (doubled=False): [27, N] -- 18 limb rows plus 9 rows of -1
    (constant companions for the on-device |r|^2 limb rows).
    rhs pattern (doubled=True): [18, N] limb rows of 2*c3.
    """
    import ml_dtypes

    src = (c3 * 2.0) if doubled else c3
    limbs = _limb_split(src)  # tuple of three [3, N] bf16
    pattern = _RHS_LIMB if doubled else _LHS_LIMB
    nrows = 18 if doubled else 27
    out = np.full((nrows, c3.shape[1]), -1.0, dtype=ml_dtypes.bfloat16)
    for k in range(3):
        for j in range(6):
            out[6 * k + j] = limbs[pattern[j]][k]
    return out


def _cand_blocks(pts):
    """[4096, 3] -> candidate-pair block arrays A, B [1024, 6]; row j of A =
    (pts[j], pts[1024+j]), row j of B = (pts[2048+j], pts[3072+j])."""
    c = pts.reshape(4, DSUB, 3)
    A = np.concatenate([c[0], c[1]], axis=1)
    Bb = np.concatenate([c[2], c[3]], axis=1)
    return (np.ascontiguousarray(A), np.ascontiguousarray(Bb))


def make_in_maps(xyz1, xyz2):
    xyz1 = np.asarray(xyz1, dtype=np.float32)
    xyz2 = np.asarray(xyz2, dtype=np.float32)
    in_maps = []
    for b in range(B):
        q3 = np.ascontiguousarray(xyz1[b].T)
        r3 = np.ascontiguousarray(xyz2[b].T)
        # [3, N] -> [48, N//16]: partition 16k+a = coord k, sixteenth a
        q3q = np.ascontiguousarray(q3.reshape(48, N // 16))
        r3q = np.ascontiguousarray(r3.reshape(48, N // 16))
        in_maps.append(
            {
                "qlh": _limb_rows(q3, doubled=False),
                "qrh": _limb_rows(q3, doubled=True),
                "rlh": _limb_rows(r3, doubled=False),
                "rrh": _limb_rows(r3, doubled=True),
                "q3": q3q,
                "r3": r3q,
                "qn": np.ascontiguousarray(xyz1[b]),
                "rn": np.ascontiguousarray(xyz2[b]),
            }
        )
        for cl, pts in (("q", xyz1[b]), ("r", xyz2[b])):
            for sfx, arr in zip("ab", _cand_blocks(pts)):
                in_maps[-1][f"{cl}b{sfx}"] = arr
    return in_maps


def unpack_outputs(results):
    d1 = np.stack([results[b]["d1"].T.reshape(-1) for b in range(B)])
    d2 = np.stack([results[b]["d2"].T.reshape(-1) for b in range(B)])
    i1 = np.stack([results[b]["i1"].T.reshape(-1) for b in range(B)])
    i2 = np.stack([results[b]["i2"].T.reshape(-1) for b in range(B)])
    return (
        d1.astype(np.float32),
        d2.astype(np.float32),
        i1.astype(np.int32),
        i2.astype(np.int32),
    )


def kernel(xyz1, xyz2):
    from concourse.bass_utils import run_bass_kernel_spmd

    nc = _get_program()
    in_maps = make_in_maps(xyz1, xyz2)
    res = run_bass_kernel_spmd(nc, in_maps, core_ids=list(range(B)))
    _CACHE["last_results"] = res
    return unpack_outputs(res.results)
